# revision 9
# baseline (speedup 1.0000x reference)
"""Causal self-attention (B=4, T=2048, C=1024, H=16, D=64) on 8 trn2 cores.

Sharding: 4-way head-parallel x 2-way batch-parallel.
  core c -> batch group bg = c // 4 (2 batches), head group hg = c % 4 (4 heads).
Each core computes q/k/v projections for its 4 heads over its 2 batches,
causal attention, and a partial output projection y_part = O @ Wo[:, cs].T.
Host sums the 4 head-group partials per batch group and adds the bias.

Per-core dataflow (all fp32 data, matmuls issued as float32r):
  qT/kT = W.T-slice.T @ xT   -> [chan, tok] in SBUF (q scaled by 1/sqrt(D))
  vT -> PE-transposed to v [tok, chan], stored per k-tile with a ones column
  S^T = kT.T @ qT per (128k x 512q) tile -> PSUM, causal tiles only
  P   = exp(S^T) on ACT (no max subtraction; scores are O(1) by construction),
        diagonal tiles masked with a triangular [128,128] strip multiply
  [O^T; rowsum] = [v|1].T @ P accumulated over k-tiles (ones row -> rowsums)
  O_norm^T = O^T * (1/rowsum) broadcast, stacked into [chan, tok] SBUF
  y_part = O_norm @ Wo_cs.T via K=128-chunk matmuls -> DRAM
"""

import numpy as np

# Full problem shape (hardcoded per the task contract).
B, T, C = 4, 2048, 1024
H, D = 16, 64
N_CORES = 8
HG = 4              # head groups (tensor parallel)
BG = 2              # batch groups (data parallel)
HPC = H // HG       # 4 heads per core
HC = HPC * D        # 256 channels per core
NB = B // BG        # 2 batches per core
P = 128
QT = 512            # q-tile width (PSUM bank limit for fp32)
KT = 128            # k-tile width (contraction tile for PV)

_CACHE: dict = {}


def _build(nb: int, t: int):
    """Build + compile the per-core Bass program. t must be divisible by 512."""
    import concourse.bass as bass
    import concourse.mybir as mybir
    import concourse.tile as tile
    from concourse import bacc
    f32 = mybir.dt.float32
    f32r = mybir.dt.float32r
    EXP = mybir.ActivationFunctionType.Exp

    KC = C // P           # 8 contraction chunks for projections
    MR = HC // P          # 2 output-channel rows for q/k/v
    nqt = t // QT         # q tiles per batch
    nkt_b = t // KT       # k tiles per batch
    tok = nb * t

    nc = bacc.Bacc("TRN2", target_bir_lowering=False, debug=False)

    xT_d = nc.dram_tensor("xT", [C, tok], f32r, kind="ExternalInput")
    wq_d = nc.dram_tensor("wq", [C, HC], f32r, kind="ExternalInput")
    wk_d = nc.dram_tensor("wk", [C, HC], f32r, kind="ExternalInput")
    wv_d = nc.dram_tensor("wv", [C, HC], f32r, kind="ExternalInput")
    wo_d = nc.dram_tensor("wo", [HC, C], f32r, kind="ExternalInput")
    mask_d = nc.dram_tensor("mask", [P, P], f32, kind="ExternalInput")
    ident_d = nc.dram_tensor("ident", [P, P], f32r, kind="ExternalInput")
    ones_d = nc.dram_tensor(
        "ones", [P, t // KT, HPC, 1], f32r, kind="ExternalInput")
    y_d = nc.dram_tensor("y", [tok, C], f32, kind="ExternalOutput")

    with tile.TileContext(nc) as tc:
        from contextlib import ExitStack

        with ExitStack() as ctx:
            const = ctx.enter_context(tc.tile_pool(name="const", bufs=1))
            bpool = ctx.enter_context(tc.tile_pool(name="bpool", bufs=1))
            xpool = ctx.enter_context(tc.tile_pool(name="xpool", bufs=3))
            vtpool = ctx.enter_context(tc.tile_pool(name="vtpool", bufs=2))
            ppool = ctx.enter_context(tc.tile_pool(name="ppool", bufs=6))
            rpool = ctx.enter_context(tc.tile_pool(name="rpool", bufs=2))
            rzpool = ctx.enter_context(tc.tile_pool(name="rzpool", bufs=2))
            rbpool = ctx.enter_context(tc.tile_pool(name="rbpool", bufs=2))
            otpool = ctx.enter_context(tc.tile_pool(name="otpool", bufs=2))
            ypool = ctx.enter_context(tc.tile_pool(name="ypool", bufs=3))
            pspool = ctx.enter_context(
                tc.tile_pool(name="pspool", bufs=2, space="PSUM"))
            spool = ctx.enter_context(
                tc.tile_pool(name="spool", bufs=2, space="PSUM"))
            opool = ctx.enter_context(
                tc.tile_pool(name="opool", bufs=2, space="PSUM"))
            trpool = ctx.enter_context(
                tc.tile_pool(name="trpool", bufs=2, space="PSUM"))

            # Constants
            wq_sb = const.tile([P, KC, HC], f32r)
            wk_sb = const.tile([P, KC, HC], f32r)
            wv_sb = const.tile([P, KC, HC], f32r)
            wo_sb = const.tile([P, MR, C], f32r)
            mask_sb = const.tile([P, P], f32)
            ident = const.tile([P, P], f32r)
            for wsb, wd in ((wq_sb, wq_d), (wk_sb, wk_d), (wv_sb, wv_d)):
                nc.sync.dma_start(
                    wsb[:], wd.ap().rearrange("(kc p) m -> p kc m", p=P))
            nc.sync.dma_start(
                wo_sb[:], wo_d.ap().rearrange("(kc p) n -> p kc n", p=P))
            nc.sync.dma_start(mask_sb[:], mask_d.ap())
            nc.sync.dma_start(ident[:], ident_d.ap())

            for b in range(nb):
                # --- Phase A: projections -> qT, kT [chan, t]; v via transpose
                qT = bpool.tile([P, MR, t], f32r, tag="qT")
                kT = bpool.tile([P, MR, t], f32r, tag="kT")
                v_sb = bpool.tile([P, nkt_b, HPC, D + 1], f32r, tag="v_sb")
                nc.sync.dma_start(v_sb[:, :, :, D:D + 1], ones_d.ap())

                for tt in range(nqt):
                    g0 = b * t + tt * QT
                    xt = xpool.tile([P, KC, QT], f32r, tag="xt")
                    nc.sync.dma_start(
                        xt[:],
                        xT_d.ap()[:, g0:g0 + QT].rearrange(
                            "(kc p) n -> p kc n", p=P))
                    vtc = vtpool.tile([P, MR, QT], f32r, tag="vtc")
                    for wsb, kind in ((wq_sb, "q"), (wk_sb, "k"), (wv_sb, "v")):
                        for m in range(MR):
                            ps = pspool.tile([P, QT], f32, tag="ps")
                            for kc in range(KC):
                                nc.tensor.matmul(
                                    ps[:],
                                    wsb[:, kc, m * P:(m + 1) * P],
                                    xt[:, kc, :],
                                    start=(kc == 0), stop=(kc == KC - 1))
                            ts = slice(tt * QT, (tt + 1) * QT)
                            if kind == "q":
                                nc.scalar.mul(qT[:, m, ts], ps[:], 1.0 / 8.0)
                            elif kind == "k":
                                nc.any.tensor_copy(kT[:, m, ts], ps[:])
                            else:
                                nc.any.tensor_copy(vtc[:, m, :], ps[:])
                    # transpose v chunk into per-k-tile [tok, chan] layout
                    for st in range(QT // P):
                        kt_g = tt * (QT // P) + st
                        for m in range(MR):
                            tp = trpool.tile([P, P], f32r, tag="tp")
                            nc.tensor.transpose(
                                tp[:], vtc[:, m, st * P:(st + 1) * P], ident[:])
                            nc.any.tensor_copy(
                                v_sb[:, kt_g, 2 * m, 0:D], tp[:, 0:D])
                            nc.any.tensor_copy(
                                v_sb[:, kt_g, 2 * m + 1, 0:D], tp[:, D:2 * D])

                # --- Phase C: causal attention per head
                o_sbk = bpool.tile([P, MR, t], f32r, tag="o_sbk")
                for h in range(HPC):
                    hm, hp = h // 2, h % 2
                    po = hp * D
                    for qt in range(nqt):
                        qb = qt * QT
                        o_ps = opool.tile([D + 1, QT], f32, tag="o_ps")
                        nkt = (qt + 1) * (QT // KT)
                        for kt in range(nkt):
                            kb = kt * KT
                            off = kb - qb
                            c0 = max(0, off)
                            s_ps = spool.tile([P, QT], f32, tag="s_ps")
                            nc.tensor.matmul(
                                s_ps[:, c0:],
                                kT[po:po + D, hm, kb:kb + KT],
                                qT[po:po + D, hm, qb + c0:qb + QT],
                                start=True, stop=True)
                            p_sb = ppool.tile([P, QT], f32r, tag="p_sb")
                            if c0:
                                nc.scalar.mul(p_sb[:, :c0], kT[:, 0, :c0], 0.0)
                            nc.scalar.activation(
                                p_sb[:, c0:], s_ps[:, c0:], EXP)
                            if off >= 0:
                                nc.vector.tensor_mul(
                                    p_sb[:, c0:c0 + P],
                                    p_sb[:, c0:c0 + P], mask_sb[:])
                            nc.tensor.matmul(
                                o_ps[:],
                                v_sb[:, kt, h, :],
                                p_sb[:],
                                start=(kt == 0), stop=(kt == nkt - 1))
                        rinv = rpool.tile([D + 1, QT], f32, tag="rinv")
                        nc.vector.reciprocal(
                            rinv[D:D + 1, :], o_ps[D:D + 1, :])
                        # partition_broadcast reads the tile's TRUE partition
                        # 0 (AP offset ignored on HW) -> DMA-shift row first.
                        rz = rzpool.tile([1, QT], f32, tag="rz")
                        nc.sync.dma_start(rz[0:1, :], rinv[D:D + 1, :])
                        rb = rbpool.tile([D, QT], f32, tag="rb")
                        nc.gpsimd.partition_broadcast(
                            rb[:], rz[0:1, :])
                        qs = slice(qb, qb + QT)
                        if hp == 0:
                            nc.vector.tensor_mul(
                                o_sbk[0:D, hm, qs], o_ps[0:D, :], rb[:])
                        else:
                            o_tmp = otpool.tile([D, QT], f32r, tag="o_tmp")
                            nc.vector.tensor_mul(
                                o_tmp[:], o_ps[0:D, :], rb[:])
                            nc.sync.dma_start(o_sbk[D:P, hm, qs], o_tmp[:])

                # --- Phase D: partial output projection
                for tt in range(t // P):
                    for n in range(C // QT):
                        yp = pspool.tile([P, QT], f32, tag="ps")
                        for kc2 in range(MR):
                            nc.tensor.matmul(
                                yp[:],
                                o_sbk[:, kc2, tt * P:(tt + 1) * P],
                                wo_sb[:, kc2, n * QT:(n + 1) * QT],
                                start=(kc2 == 0), stop=(kc2 == MR - 1))
                        ysb = ypool.tile([P, QT], f32, tag="ysb")
                        nc.any.tensor_copy(ysb[:], yp[:])
                        nc.sync.dma_start(
                            y_d.ap()[b * t + tt * P:b * t + (tt + 1) * P,
                                     n * QT:(n + 1) * QT],
                            ysb[:])

    nc.compile()
    return nc


def _aux_inputs(t):
    return {
        "mask": np.ascontiguousarray(np.triu(np.ones((P, P), np.float32))),
        "ident": np.ascontiguousarray(np.eye(P, dtype=np.float32)),
        "ones": np.ones((P, t // KT, HPC, 1), dtype=np.float32),
    }


def _make_in_maps(x, Wq, Wk, Wv, Wo):
    aux = _aux_inputs(T)
    in_maps = []
    for c in range(N_CORES):
        bg, hg = c // HG, c % HG
        cs = slice(hg * HC, (hg + 1) * HC)
        xT = np.ascontiguousarray(
            x[bg * NB:(bg + 1) * NB].reshape(NB * T, C).T)
        in_maps.append({
            "xT": xT,
            "wq": np.ascontiguousarray(Wq[cs, :].T),
            "wk": np.ascontiguousarray(Wk[cs, :].T),
            "wv": np.ascontiguousarray(Wv[cs, :].T),
            "wo": np.ascontiguousarray(Wo[:, cs].T),
            **aux,
        })
    return in_maps


def _reduce_out(ys, bo):
    out = np.empty((B, T, C), dtype=np.float32)
    for bg in range(BG):
        acc = ys[bg * HG].astype(np.float32).copy()
        for hg in range(1, HG):
            acc += ys[bg * HG + hg]
        acc += bo[None, :]
        out[bg * NB:(bg + 1) * NB] = acc.reshape(NB, T, C)
    return out


def _ensure_ntff_hook():
    """The agent image's antenv lacks axon_hooks; synthesize it from the
    boot module's ctypes hook so trace=True captures NTFF profiles."""
    import sys
    import types

    try:
        from antenv.axon_hooks import get_axon_ntff_profile_hook  # noqa: F401
        return
    except ImportError:
        pass
    if "/root/.axon_site" not in sys.path:
        sys.path.insert(0, "/root/.axon_site")
    try:
        from trn_agent_boot.trn_boot import _ntff_profile_via_ctypes

        hook = _ntff_profile_via_ctypes("/opt/axon/libaxon_pjrt.so")
    except Exception:
        hook = None
    mod = types.ModuleType("antenv.axon_hooks")
    mod.get_axon_ntff_profile_hook = lambda: hook
    mod.set_axon_ntff_profile_hook = lambda h: None
    import antenv

    sys.modules["antenv.axon_hooks"] = mod
    antenv.axon_hooks = mod


def run(x, Wq, Wk, Wv, Wo, bo, trace=False):
    """Run on HW; returns (out, BassKernelResults)."""
    from concourse.bass_utils import run_bass_kernel_spmd

    if trace:
        _ensure_ntff_hook()

    x = np.asarray(x, dtype=np.float32)
    Wq = np.asarray(Wq, dtype=np.float32)
    Wk = np.asarray(Wk, dtype=np.float32)
    Wv = np.asarray(Wv, dtype=np.float32)
    Wo = np.asarray(Wo, dtype=np.float32)
    bo = np.asarray(bo, dtype=np.float32)

    if "nc" not in _CACHE:
        _CACHE["nc"] = _build(NB, T)
    nc = _CACHE["nc"]
    in_maps = _make_in_maps(x, Wq, Wk, Wv, Wo)
    rr = run_bass_kernel_spmd(
        nc, in_maps, core_ids=list(range(N_CORES)), trace=trace)
    ys = [r["y"] for r in rr.results]
    return _reduce_out(ys, bo), rr


def kernel(x, Wq, Wk, Wv, Wo, bo):
    out, _ = run(x, Wq, Wk, Wv, Wo, bo, trace=False)
    return out


if __name__ == "__main__":
    # Tiny-config CoreSim check: 1 batch, t=512 -> 1 q-tile, 4 k-tiles.
    import sys

    t_small = 512
    nb_small = 1
    nc = _build(nb_small, t_small)

    rng = np.random.default_rng(0)
    tok = nb_small * t_small
    x = rng.standard_normal((tok, C), dtype=np.float32)
    sc = 1.0 / np.sqrt(C)
    Wq = rng.standard_normal((C, C), dtype=np.float32) * sc
    Wk = rng.standard_normal((C, C), dtype=np.float32) * sc
    Wv = rng.standard_normal((C, C), dtype=np.float32) * sc
    Wo = rng.standard_normal((C, C), dtype=np.float32) * sc

    hg = 1
    cs = slice(hg * HC, (hg + 1) * HC)
    mask = np.ascontiguousarray(np.triu(np.ones((P, P), dtype=np.float32)))

    from concourse.bass_interp import CoreSim

    sim = CoreSim(nc, trace=False)
    sim.tensor("xT")[:] = np.ascontiguousarray(x.T)
    sim.tensor("wq")[:] = np.ascontiguousarray(Wq[cs, :].T)
    sim.tensor("wk")[:] = np.ascontiguousarray(Wk[cs, :].T)
    sim.tensor("wv")[:] = np.ascontiguousarray(Wv[cs, :].T)
    sim.tensor("wo")[:] = np.ascontiguousarray(Wo[:, cs].T)
    sim.tensor("mask")[:] = mask
    aux = _aux_inputs(t_small)
    sim.tensor("ident")[:] = aux["ident"]
    sim.tensor("ones")[:] = aux["ones"]
    print("simulating...", flush=True)
    sim.simulate()
    y_hw = np.array(sim.tensor("y"))

    # numpy reference for the per-core partial
    q = (x @ Wq.T)[:, cs]
    k = (x @ Wk.T)[:, cs]
    v = (x @ Wv.T)[:, cs]
    y_ref = np.zeros((tok, C), dtype=np.float32)
    for h in range(HPC):
        ds = slice(h * D, (h + 1) * D)
        s = (q[:, ds] @ k[:, ds].T) / np.sqrt(D)
        causal = np.tril(np.ones((tok, tok), dtype=bool))
        s = np.where(causal, s, -np.inf)
        p = np.exp(s - s.max(axis=-1, keepdims=True))
        p /= p.sum(axis=-1, keepdims=True)
        o = p @ v[:, ds]
        y_ref += o @ Wo[:, cs][:, ds].T
    err = np.abs(y_hw - y_ref).max() / np.abs(y_ref).max()
    print("sim rel err:", err)
    sys.exit(0 if err < 1e-3 else 1)
